# revision 15
# baseline (speedup 1.0000x reference)
"""AnomalyAttention distributed Bass kernel for 8 TRN2 NeuronCores.

Problem: B=4, L=1024, H=8, E=64. Outputs (V, series, prior, sigma4).
Sharding: data-parallel over the 32 (b,h) pairs -> 4 pairs per core, no
communication. Host pre-transposes Q/K to [E, L] per pair so every device
DMA is contiguous.

Per (b,h) pair, per 128-row query chunk m (w = 128*(m+1) unmasked cols):
  diag bank of psum preloaded with -1e9 causal mask via identity matmul
  scores[:, :w] = qT_chunk.T @ kT[:, :w] (accumulated on mask)   (PE, K=64)
  P[:, :w] = exp(0.125*scores), rowsum via accum_out             (ACT)
  S = P * (1/rowsum) -> series DMA (masked tail stays zero: outputs
    are donated pre-zeroed buffers)
  seriesT blocks via PE transpose of unnormalized P -> V matmul,
    normalization folded into the PSUM->SBUF copy                (PE/DVE)
  sq = (col - i)^2 = Square(colf, bias=-i)                       (ACT)
  prior = exp(c1*sq) * c2                                        (ACT + DVE)
  sigma4 = broadcast(s2)                                         (GPSIMD)

TRN2 constraint: a Matmult can carry at most one sync wait, so each DMA'd
input gets a tiny "absorber" matmul at pair start and PSUM tiles keep a
single-engine reader set.
"""

import math
from contextlib import ExitStack

import numpy as np

import concourse.bass as bass
import concourse.mybir as mybir
import concourse.tile as tile
from concourse import bacc
from concourse.bass_utils import run_bass_kernel_spmd
from concourse.masks import make_causal_mask, make_identity

B, L, H, E = 4, 1024, 8, 64
N_CORES = 8
PAIRS = (B * H) // N_CORES  # 4 (b,h) pairs per core
PCH = 128                   # query-chunk rows (partition dim)
NCH = L // PCH              # 8 chunks
SCALE = 1.0 / math.sqrt(E)
LN3 = math.log(3.0)
INV_SQRT_2PI = 1.0 / math.sqrt(2.0 * math.pi)
NEG_INF = -1e9
F32 = mybir.dt.float32
I32 = mybir.dt.int32

_NC_CACHE = []


def _build():
    nc = bacc.Bacc()

    qkt_d = nc.dram_tensor("qkt", [PAIRS, E, 2, L], F32, kind="ExternalInput")
    v_d = nc.dram_tensor("v", [PAIRS, L, E], F32, kind="ExternalInput")
    sig_d = nc.dram_tensor("sig", [PAIRS, L], F32, kind="ExternalInput")
    series_d = nc.dram_tensor("series", [PAIRS, L, L], F32, kind="ExternalOutput")
    prior_d = nc.dram_tensor("prior", [PAIRS, L, L], F32, kind="ExternalOutput")
    sigma4_d = nc.dram_tensor("sigma4", [PAIRS, L, L], F32, kind="ExternalOutput")
    vout_d = nc.dram_tensor("vout", [PAIRS, L, E], F32, kind="ExternalOutput")

    with tile.TileContext(nc) as tc, ExitStack() as ctx:
        consts = ctx.enter_context(tc.tile_pool(name="consts", bufs=1))
        io_qk = ctx.enter_context(tc.tile_pool(name="io_qk", bufs=2))
        io_small = ctx.enter_context(tc.tile_pool(name="io_small", bufs=2))
        sigp = ctx.enter_context(tc.tile_pool(name="sigp", bufs=2))
        big = ctx.enter_context(tc.tile_pool(name="big", bufs=3))
        smalls = ctx.enter_context(tc.tile_pool(name="smalls", bufs=4))
        strans = ctx.enter_context(tc.tile_pool(name="strans", bufs=NCH + 1))
        vsb = ctx.enter_context(tc.tile_pool(name="vsb", bufs=2))
        ps_scores = ctx.enter_context(tc.tile_pool(name="ps_scores", bufs=2, space="PSUM"))
        ps_t = ctx.enter_context(tc.tile_pool(name="ps_t", bufs=2, space="PSUM"))
        ps_v = ctx.enter_context(tc.tile_pool(name="ps_v", bufs=1, space="PSUM"))

        ident = consts.tile([128, 128], F32)
        make_identity(nc, ident)
        ident1 = consts.tile([1, 1], F32)
        nc.vector.memset(ident1, 1.0)
        maskneg = consts.tile([128, 128], F32)
        make_causal_mask(nc, maskneg, mask_val=NEG_INF)
        coli = consts.tile([128, L], I32)
        nc.gpsimd.iota(coli, [[1, L]], channel_multiplier=0)
        colf = consts.tile([128, L], F32)
        nc.vector.tensor_copy(colf, coli)
        negi_i = consts.tile([128, NCH], I32)
        nc.gpsimd.iota(negi_i, [[-PCH, NCH]], channel_multiplier=-1)
        negi = consts.tile([128, NCH], F32)
        nc.vector.tensor_copy(negi, negi_i)
        ones = consts.tile([128, L], F32)
        nc.vector.memset(ones, 1.0)
        eps_b = consts.tile([128, 1], F32)
        nc.vector.memset(eps_b, LN3 * 1e-5)

        # Never-read PSUM scratch for DMA-wait-absorber matmuls. Allocated
        # once so its reuse never creates cross-engine WAR waits.
        junk = ps_v.tile([1, 3 * PAIRS], F32, tag="junk")

        for p in range(PAIRS):
            qkt_sb = io_qk.tile([E, 2, L], F32, tag="qkt_sb")
            nc.sync.dma_start(out=qkt_sb, in_=qkt_d[p])
            qt_sb = qkt_sb[:, 0, :]
            kt_sb = qkt_sb[:, 1, :]
            v_sb = io_small.tile([128, NCH, E], F32, tag="v_sb")
            nc.sync.dma_start(
                out=v_sb, in_=v_d[p].rearrange("(n pp) d -> pp n d", pp=PCH)
            )
            sig_row = io_small.tile([1, L], F32, tag="sig_row")
            nc.sync.dma_start(out=sig_row, in_=sig_d[p : p + 1, :])

            # Wait-absorbers: one tiny matmul per DMA'd input so real matmuls
            # never carry a DMA wait (Matmult has a single sync-wait slot).
            nc.tensor.matmul(
                junk[:, 3 * p : 3 * p + 1], qkt_sb[0:1, 0, 0:1],
                qkt_sb[0:1, 0, 0:1], start=True, stop=True,
            )
            nc.tensor.matmul(
                junk[:, 3 * p + 1 : 3 * p + 2], v_sb[0:1, 0, 0:1],
                v_sb[0:1, 0, 0:1], start=True, stop=True,
            )
            nc.tensor.matmul(
                junk[:, 3 * p + 2 : 3 * p + 3], sig_row[0:1, 0:1],
                sig_row[0:1, 0:1], start=True, stop=True,
            )

            # sigma [1, L] -> [128, NCH] column layout via PE transposes.
            psum_sig = ps_t.tile([128, NCH], F32, tag="psum_t")
            for m in range(NCH):
                nc.tensor.transpose(
                    psum_sig[:, m : m + 1],
                    sig_row[:, m * PCH : (m + 1) * PCH],
                    ident1,
                )
            sig_t = sigp.tile([128, NCH], F32, tag="sig_t")
            nc.vector.tensor_copy(sig_t, psum_sig)

            # sigma chain: s2 = 3^(sigmoid(5x) + 1e-5) - 1
            s2 = sigp.tile([128, NCH], F32, tag="s2")
            nc.scalar.activation(
                s2, sig_t, mybir.ActivationFunctionType.Sigmoid, scale=5.0
            )
            nc.scalar.activation(
                s2, s2, mybir.ActivationFunctionType.Exp, scale=LN3, bias=eps_b
            )
            nc.vector.tensor_scalar_add(s2, s2, -1.0)
            # c1 = -1/(2*s2^2), c2 = 1/(sqrt(2pi)*s2)
            sqs = sigp.tile([128, NCH], F32, tag="sqs")
            nc.vector.tensor_mul(sqs, s2, s2)
            c1 = sigp.tile([128, NCH], F32, tag="c1")
            nc.vector.reciprocal(c1, sqs)
            nc.vector.tensor_scalar_mul(c1, c1, -0.5)
            c2 = sigp.tile([128, NCH], F32, tag="c2")
            nc.vector.reciprocal(c2, s2)
            nc.vector.tensor_scalar_mul(c2, c2, INV_SQRT_2PI)

            vout_sb = vsb.tile([128, NCH, E], F32, tag="vout_sb")

            for m in range(NCH):
                w = PCH * (m + 1)
                # ---- scores (mask preloaded into the diagonal bank) ----
                psum_s = ps_scores.tile([128, L], F32, tag="psum_s")
                nc.tensor.matmul(
                    psum_s[:, w - PCH : w], ident, maskneg, start=True, stop=False
                )
                blocks = [(j0, min(512, w - j0)) for j0 in range(0, w, 512)]
                # diag-containing block first so its accumulation group stays
                # contiguous with the mask matmul
                for j0, nn in sorted(blocks, key=lambda b: -(b[0] + b[1])):
                    has_diag = j0 + nn == w
                    nc.tensor.matmul(
                        psum_s[:, j0 : j0 + nn],
                        qt_sb[:, m * PCH : (m + 1) * PCH],
                        kt_sb[:, j0 : j0 + nn],
                        start=not has_diag,
                        stop=True,
                    )
                # ---- softmax ----
                p_tile = big.tile([128, L], F32, tag="p_tile")
                rowsum = smalls.tile([128, 1], F32, tag="rowsum")
                nc.scalar.activation(
                    p_tile[:, 0:w],
                    psum_s[:, 0:w],
                    mybir.ActivationFunctionType.Exp,
                    scale=SCALE,
                    accum_out=rowsum,
                )
                recip = smalls.tile([128, 1], F32, tag="recip")
                nc.vector.reciprocal(recip, rowsum)
                s_tile = big.tile([128, L], F32, tag="s_tile")
                nc.vector.tensor_scalar_mul(
                    s_tile[:, 0:w], p_tile[:, 0:w], recip
                )
                nc.sync.dma_start(
                    out=series_d[p, m * PCH : (m + 1) * PCH, 0:w],
                    in_=s_tile[:, 0:w],
                )
                # ---- prior ----
                sq_t = big.tile([128, L], F32, tag="sq_t")
                nc.scalar.activation(
                    sq_t,
                    colf,
                    mybir.ActivationFunctionType.Square,
                    bias=negi[:, m : m + 1],
                )
                pr_tile = big.tile([128, L], F32, tag="pr_tile")
                nc.scalar.activation(
                    pr_tile,
                    sq_t,
                    mybir.ActivationFunctionType.Exp,
                    scale=c1[:, m : m + 1],
                )
                nc.vector.tensor_scalar_mul(pr_tile, pr_tile, c2[:, m : m + 1])
                nc.sync.dma_start(
                    out=prior_d[p, m * PCH : (m + 1) * PCH, :], in_=pr_tile
                )
                # ---- sigma4 ----
                g_tile = big.tile([128, L], F32, tag="g_tile")
                nc.gpsimd.tensor_scalar_mul(g_tile, ones, s2[:, m : m + 1])
                nc.sync.dma_start(
                    out=sigma4_d[p, m * PCH : (m + 1) * PCH, :], in_=g_tile
                )
                # ---- V = series @ values (unnormalized; fold 1/rowsum into
                # the PSUM->SBUF copy) ----
                psum_v = ps_v.tile([128, E], F32, tag="psum_v")
                sts = []
                for n in range(m + 1):
                    psum_t = ps_t.tile([128, 128], F32, tag="psum_t")
                    nc.tensor.transpose(
                        psum_t, p_tile[:, n * PCH : (n + 1) * PCH], ident
                    )
                    st = strans.tile([128, 128], F32, tag="st")
                    nc.vector.tensor_copy(st, psum_t)
                    sts.append(st)
                for n in range(m + 1):
                    nc.tensor.matmul(
                        psum_v,
                        sts[n],
                        v_sb[:, n, :],
                        start=(n == 0),
                        stop=(n == m),
                    )
                nc.vector.tensor_scalar_mul(vout_sb[:, m, :], psum_v, recip)

            nc.sync.dma_start(
                out=vout_d[p].rearrange("(n pp) d -> pp n d", pp=PCH), in_=vout_sb
            )

    nc.compile()
    return nc


def _get_nc():
    if not _NC_CACHE:
        _NC_CACHE.append(_build())
    return _NC_CACHE[0]


def kernel(**inputs) -> tuple[np.ndarray, np.ndarray, np.ndarray, np.ndarray]:
    queries = np.asarray(inputs["queries"], dtype=np.float32)
    keys = np.asarray(inputs["keys"], dtype=np.float32)
    values = np.asarray(inputs["values"], dtype=np.float32)
    sigma = np.asarray(inputs["sigma"], dtype=np.float32)

    in_maps = []
    for c in range(N_CORES):
        qkts, vs, sigs = [], [], []
        for k in range(PAIRS):
            b, h = divmod(PAIRS * c + k, H)
            qkts.append(
                np.ascontiguousarray(
                    np.stack(
                        [queries[b, :, h, :].T, keys[b, :, h, :].T], axis=1
                    )
                )
            )
            vs.append(np.ascontiguousarray(values[b, :, h, :]))
            sigs.append(np.ascontiguousarray(sigma[b, :, h]))
        in_maps.append(
            {
                "qkt": np.stack(qkts),
                "v": np.stack(vs),
                "sig": np.stack(sigs),
            }
        )

    nc = _get_nc()
    import os as _os

    _kw = {}
    if _os.environ.get("BASS_TMPDIR"):
        _kw["tmpdir"] = _os.environ["BASS_TMPDIR"]
    res = run_bass_kernel_spmd(nc, in_maps, core_ids=list(range(N_CORES)), **_kw)
    kernel.last_results = res

    V = np.empty((B, L, H, E), np.float32)
    series = np.empty((B, H, L, L), np.float32)
    prior = np.empty((B, H, L, L), np.float32)
    sigma4 = np.empty((B, H, L, L), np.float32)
    for c in range(N_CORES):
        r = res.results[c]
        for k in range(PAIRS):
            b, h = divmod(PAIRS * c + k, H)
            V[b, :, h, :] = r["vout"][k]
            series[b, h] = r["series"][k]
            prior[b, h] = r["prior"][k]
            sigma4[b, h] = r["sigma4"][k]
    return V, series, prior, sigma4


kernel.last_results = None


# revision 16
# speedup vs baseline: 2.1846x; 2.1846x over previous
"""AnomalyAttention distributed Bass kernel for 8 TRN2 NeuronCores.

Problem: B=4, L=1024, H=8, E=64. Outputs (V, series, prior, sigma4).
Sharding: data-parallel over the 32 (b,h) pairs -> 4 pairs per core, no
communication. Host pre-transposes Q/K to [E, L] per pair so every device
DMA is contiguous.

Per (b,h) pair, per 128-row query chunk m (w = 128*(m+1) unmasked cols):
  diag bank of psum preloaded with -1e9 causal mask via identity matmul
  scores[:, :w] = qT_chunk.T @ kT[:, :w] (accumulated on mask)   (PE, K=64)
  P[:, :w] = exp(0.125*scores), rowsum via accum_out             (ACT)
  S = P * (1/rowsum) -> series DMA (masked tail stays zero: outputs
    are donated pre-zeroed buffers)
  seriesT blocks via PE transpose of unnormalized P -> V matmul,
    normalization folded into the PSUM->SBUF copy                (PE/DVE)
  sq = (col - i)^2 = Square(colf, bias=-i)                       (ACT)
  prior = exp(c1*sq) * c2                                        (ACT + DVE)
  sigma4 = broadcast(s2)                                         (GPSIMD)

TRN2 constraint: a Matmult can carry at most one sync wait, so each DMA'd
input gets a tiny "absorber" matmul at pair start and PSUM tiles keep a
single-engine reader set.
"""

import math
from contextlib import ExitStack

import numpy as np

import concourse.bass as bass
import concourse.mybir as mybir
import concourse.tile as tile
from concourse import bacc
from concourse.bass_utils import run_bass_kernel_spmd
from concourse.masks import make_causal_mask, make_identity

B, L, H, E = 4, 1024, 8, 64
N_CORES = 8
PAIRS = (B * H) // N_CORES  # 4 (b,h) pairs per core
PCH = 128                   # query-chunk rows (partition dim)
NCH = L // PCH              # 8 chunks
SCALE = 1.0 / math.sqrt(E)
LN3 = math.log(3.0)
INV_SQRT_2PI = 1.0 / math.sqrt(2.0 * math.pi)
NEG_INF = -1e9
F32 = mybir.dt.float32
I32 = mybir.dt.int32

_NC_CACHE = []


def _build():
    nc = bacc.Bacc()

    qkt_d = nc.dram_tensor("qkt", [PAIRS, E, 2, L], F32, kind="ExternalInput")
    v_d = nc.dram_tensor("v", [PAIRS, L, E], F32, kind="ExternalInput")
    sig_d = nc.dram_tensor("sig", [PAIRS, L], F32, kind="ExternalInput")
    series_d = nc.dram_tensor("series", [PAIRS, L, L], F32, kind="ExternalOutput")
    prior_d = nc.dram_tensor("prior", [PAIRS, L, L], F32, kind="ExternalOutput")
    sigma4_d = nc.dram_tensor("sigma4", [PAIRS, L, L], F32, kind="ExternalOutput")
    vout_d = nc.dram_tensor("vout", [PAIRS, L, E], F32, kind="ExternalOutput")

    with tile.TileContext(nc) as tc, ExitStack() as ctx:
        consts = ctx.enter_context(tc.tile_pool(name="consts", bufs=1))
        io_qk = ctx.enter_context(tc.tile_pool(name="io_qk", bufs=2))
        io_small = ctx.enter_context(tc.tile_pool(name="io_small", bufs=2))
        sigp = ctx.enter_context(tc.tile_pool(name="sigp", bufs=2))
        big = ctx.enter_context(tc.tile_pool(name="big", bufs=3))
        smalls = ctx.enter_context(tc.tile_pool(name="smalls", bufs=4))
        strans = ctx.enter_context(tc.tile_pool(name="strans", bufs=NCH + 1))
        vsb = ctx.enter_context(tc.tile_pool(name="vsb", bufs=2))
        ps_scores = ctx.enter_context(tc.tile_pool(name="ps_scores", bufs=2, space="PSUM"))
        ps_t = ctx.enter_context(tc.tile_pool(name="ps_t", bufs=2, space="PSUM"))
        ps_v = ctx.enter_context(tc.tile_pool(name="ps_v", bufs=1, space="PSUM"))

        ident = consts.tile([128, 128], F32)
        make_identity(nc, ident)
        ident1 = consts.tile([1, 1], F32)
        nc.vector.memset(ident1, 1.0)
        maskneg = consts.tile([128, 128], F32)
        make_causal_mask(nc, maskneg, mask_val=NEG_INF)
        coli = consts.tile([128, L], I32)
        nc.gpsimd.iota(coli, [[1, L]], channel_multiplier=0)
        colf = consts.tile([128, L], F32)
        nc.vector.tensor_copy(colf, coli)
        negi_i = consts.tile([128, NCH], I32)
        nc.gpsimd.iota(negi_i, [[-PCH, NCH]], channel_multiplier=-1)
        negi = consts.tile([128, NCH], F32)
        nc.vector.tensor_copy(negi, negi_i)
        ones = consts.tile([128, L], F32)
        nc.vector.memset(ones, 1.0)
        eps_b = consts.tile([128, 1], F32)
        nc.vector.memset(eps_b, LN3 * 1e-5)

        # Never-read PSUM scratch for DMA-wait-absorber matmuls. Allocated
        # once so its reuse never creates cross-engine WAR waits.
        junk = ps_v.tile([1, 3 * PAIRS], F32, tag="junk")

        for p in range(PAIRS):
            qkt_sb = io_qk.tile([E, 2, L], F32, tag="qkt_sb")
            nc.sync.dma_start(out=qkt_sb, in_=qkt_d[p])
            qt_sb = qkt_sb[:, 0, :]
            kt_sb = qkt_sb[:, 1, :]
            v_sb = io_small.tile([128, NCH, E], F32, tag="v_sb")
            nc.sync.dma_start(
                out=v_sb, in_=v_d[p].rearrange("(n pp) d -> pp n d", pp=PCH)
            )
            sig_row = io_small.tile([1, L], F32, tag="sig_row")
            nc.sync.dma_start(out=sig_row, in_=sig_d[p : p + 1, :])

            # Wait-absorbers: one tiny matmul per DMA'd input so real matmuls
            # never carry a DMA wait (Matmult has a single sync-wait slot).
            nc.tensor.matmul(
                junk[:, 3 * p : 3 * p + 1], qkt_sb[0:1, 0, 0:1],
                qkt_sb[0:1, 0, 0:1], start=True, stop=True,
            )
            nc.tensor.matmul(
                junk[:, 3 * p + 1 : 3 * p + 2], v_sb[0:1, 0, 0:1],
                v_sb[0:1, 0, 0:1], start=True, stop=True,
            )
            nc.tensor.matmul(
                junk[:, 3 * p + 2 : 3 * p + 3], sig_row[0:1, 0:1],
                sig_row[0:1, 0:1], start=True, stop=True,
            )

            # sigma [1, L] -> [128, NCH] column layout via PE transposes.
            psum_sig = ps_t.tile([128, NCH], F32, tag="psum_t")
            for m in range(NCH):
                nc.tensor.transpose(
                    psum_sig[:, m : m + 1],
                    sig_row[:, m * PCH : (m + 1) * PCH],
                    ident1,
                )
            sig_t = sigp.tile([128, NCH], F32, tag="sig_t")
            nc.vector.tensor_copy(sig_t, psum_sig)

            # sigma chain: s2 = 3^(sigmoid(5x) + 1e-5) - 1
            s2 = sigp.tile([128, NCH], F32, tag="s2")
            nc.scalar.activation(
                s2, sig_t, mybir.ActivationFunctionType.Sigmoid, scale=5.0
            )
            nc.scalar.activation(
                s2, s2, mybir.ActivationFunctionType.Exp, scale=LN3, bias=eps_b
            )
            nc.vector.tensor_scalar_add(s2, s2, -1.0)
            # c1 = -1/(2*s2^2), c2 = 1/(sqrt(2pi)*s2)
            sqs = sigp.tile([128, NCH], F32, tag="sqs")
            nc.vector.tensor_mul(sqs, s2, s2)
            c1 = sigp.tile([128, NCH], F32, tag="c1")
            nc.vector.reciprocal(c1, sqs)
            nc.vector.tensor_scalar_mul(c1, c1, -0.5)
            c2 = sigp.tile([128, NCH], F32, tag="c2")
            nc.vector.reciprocal(c2, s2)
            nc.vector.tensor_scalar_mul(c2, c2, INV_SQRT_2PI)

            vout_sb = vsb.tile([128, NCH, E], F32, tag="vout_sb")

            for m in range(NCH):
                w = PCH * (m + 1)
                # ---- scores (mask preloaded into the diagonal bank) ----
                psum_s = ps_scores.tile([128, L], F32, tag="psum_s")
                nc.tensor.matmul(
                    psum_s[:, w - PCH : w], ident, maskneg, start=True, stop=False
                )
                blocks = [(j0, min(512, w - j0)) for j0 in range(0, w, 512)]
                # diag-containing block first so its accumulation group stays
                # contiguous with the mask matmul
                for j0, nn in sorted(blocks, key=lambda b: -(b[0] + b[1])):
                    has_diag = j0 + nn == w
                    nc.tensor.matmul(
                        psum_s[:, j0 : j0 + nn],
                        qt_sb[:, m * PCH : (m + 1) * PCH],
                        kt_sb[:, j0 : j0 + nn],
                        start=not has_diag,
                        stop=True,
                    )
                # ---- softmax ----
                p_tile = big.tile([128, L], F32, tag="p_tile")
                rowsum = smalls.tile([128, 1], F32, tag="rowsum")
                nc.scalar.activation(
                    p_tile[:, 0:w],
                    psum_s[:, 0:w],
                    mybir.ActivationFunctionType.Exp,
                    scale=SCALE,
                    accum_out=rowsum,
                )
                recip = smalls.tile([128, 1], F32, tag="recip")
                nc.vector.reciprocal(recip, rowsum)
                s_tile = big.tile([128, L], F32, tag="s_tile")
                nc.vector.tensor_scalar_mul(
                    s_tile[:, 0:w], p_tile[:, 0:w], recip
                )
                nc.sync.dma_start(
                    out=series_d[p, m * PCH : (m + 1) * PCH, 0:w],
                    in_=s_tile[:, 0:w],
                )
                # ---- prior ----
                sq_t = big.tile([128, L], F32, tag="sq_t")
                nc.scalar.activation(
                    sq_t,
                    colf,
                    mybir.ActivationFunctionType.Square,
                    bias=negi[:, m : m + 1],
                )
                pr_tile = big.tile([128, L], F32, tag="pr_tile")
                nc.scalar.activation(
                    pr_tile,
                    sq_t,
                    mybir.ActivationFunctionType.Exp,
                    scale=c1[:, m : m + 1],
                )
                nc.vector.tensor_scalar_mul(pr_tile, pr_tile, c2[:, m : m + 1])
                nc.sync.dma_start(
                    out=prior_d[p, m * PCH : (m + 1) * PCH, :], in_=pr_tile
                )
                # ---- sigma4 ----
                g_tile = big.tile([128, L], F32, tag="g_tile")
                nc.scalar.mul(g_tile, ones, mul=s2[:, m : m + 1])
                nc.sync.dma_start(
                    out=sigma4_d[p, m * PCH : (m + 1) * PCH, :], in_=g_tile
                )
                # ---- V = series @ values (unnormalized; fold 1/rowsum into
                # the PSUM->SBUF copy) ----
                psum_v = ps_v.tile([128, E], F32, tag="psum_v")
                sts = []
                for n in range(m + 1):
                    psum_t = ps_t.tile([128, 128], F32, tag="psum_t")
                    nc.tensor.transpose(
                        psum_t, p_tile[:, n * PCH : (n + 1) * PCH], ident
                    )
                    st = strans.tile([128, 128], F32, tag="st")
                    nc.vector.tensor_copy(st, psum_t)
                    sts.append(st)
                for n in range(m + 1):
                    nc.tensor.matmul(
                        psum_v,
                        sts[n],
                        v_sb[:, n, :],
                        start=(n == 0),
                        stop=(n == m),
                    )
                nc.vector.tensor_scalar_mul(vout_sb[:, m, :], psum_v, recip)

            nc.sync.dma_start(
                out=vout_d[p].rearrange("(n pp) d -> pp n d", pp=PCH), in_=vout_sb
            )

    nc.compile()
    return nc


def _get_nc():
    if not _NC_CACHE:
        _NC_CACHE.append(_build())
    return _NC_CACHE[0]


def kernel(**inputs) -> tuple[np.ndarray, np.ndarray, np.ndarray, np.ndarray]:
    queries = np.asarray(inputs["queries"], dtype=np.float32)
    keys = np.asarray(inputs["keys"], dtype=np.float32)
    values = np.asarray(inputs["values"], dtype=np.float32)
    sigma = np.asarray(inputs["sigma"], dtype=np.float32)

    in_maps = []
    for c in range(N_CORES):
        qkts, vs, sigs = [], [], []
        for k in range(PAIRS):
            b, h = divmod(PAIRS * c + k, H)
            qkts.append(
                np.ascontiguousarray(
                    np.stack(
                        [queries[b, :, h, :].T, keys[b, :, h, :].T], axis=1
                    )
                )
            )
            vs.append(np.ascontiguousarray(values[b, :, h, :]))
            sigs.append(np.ascontiguousarray(sigma[b, :, h]))
        in_maps.append(
            {
                "qkt": np.stack(qkts),
                "v": np.stack(vs),
                "sig": np.stack(sigs),
            }
        )

    nc = _get_nc()
    import os as _os

    _kw = {}
    if _os.environ.get("BASS_TMPDIR"):
        _kw["tmpdir"] = _os.environ["BASS_TMPDIR"]
    res = run_bass_kernel_spmd(nc, in_maps, core_ids=list(range(N_CORES)), **_kw)
    kernel.last_results = res

    V = np.empty((B, L, H, E), np.float32)
    series = np.empty((B, H, L, L), np.float32)
    prior = np.empty((B, H, L, L), np.float32)
    sigma4 = np.empty((B, H, L, L), np.float32)
    for c in range(N_CORES):
        r = res.results[c]
        for k in range(PAIRS):
            b, h = divmod(PAIRS * c + k, H)
            V[b, :, h, :] = r["vout"][k]
            series[b, h] = r["series"][k]
            prior[b, h] = r["prior"][k]
            sigma4[b, h] = r["sigma4"][k]
    return V, series, prior, sigma4


kernel.last_results = None
